# revision 32
# baseline (speedup 1.0000x reference)
"""Trainium2 Bass kernel for a single-head transformer encoder layer with
gumbel-softmax attention.

Reference computation (per batch):
    q,k,v = src@wq+bq, src@wk+bk, src@wv+bv
    attn  = softmax(q@k.T/sqrt(D) + (-log(-log(u))))
    x     = LN1(src + attn@v)
    out   = LN2(x + gelu(x@w1+b1)@w2 + b2)

Sharding: pure data-parallel over batch. B=16 over 8 cores -> 2 batches/core.

v5 design:
  - scores computed TRANSPOSED (S^T[k,q] = k @ q^T per 128-k block) so the
    post-exp matrix is already in the [k-part, q-free] stationary layout the
    PV matmul wants -> no P transposes. Gumbel noise ln(-ln u) is
    host-precomputed (fp16, transposed) -> no ACT Ln passes.
  - softmax denominator z rides a ones-column appended to v (v_ext[:,:,768])
    so PV emits [q, 384|384+1] and z lands per-q-partition.
  - QK projection fp8 DoubleRow; FFN2 fp8 DoubleRow; V/PV/FFN1 bf16.
  - ln1_w/ln1_b folded into w1/b1 on host (exact): FFN1 consumes the raw
    LN1 output y; x=y*w+b (+b2, host-folded) is only materialized for the
    LN2 residual, on GpSimd, off the critical path.
  - the structural transpose (y [q,e] -> yT [e,q] for FFN1) is one XBAR DMA
    transpose per q-block writing the strided yT slice directly.
  - LN stats come from accum_out side-channels (residual stt emits sum(x),
    one ACT Square pass emits sum(x^2)) -> no bn_stats chains; the LN apply
    for block qc-1 is emitted during block qc so the ACT-computed rstd is
    never waited on in-queue.
  - PSUM tiles are [128,2,512] pairs so most evacuations cover two
    accumulation groups in one instruction.
  - ACT table sets phase-separated: attention+LN use natural_log_exp set
    (Square/Copy/Identity are in every set), FFN1 gelu evacs batch together.
"""

import numpy as np
import ml_dtypes

import bass_rust
import concourse.bass as bass
import concourse.tile as tile
from concourse import mybir
from concourse.vector_clock import ScopedClock
from concourse.bass_utils import run_bass_kernel_spmd

FP32 = mybir.dt.float32
FP16 = mybir.dt.float16
BF16 = mybir.dt.bfloat16
F8 = mybir.dt.float8e4
AF = mybir.ActivationFunctionType
OP = mybir.AluOpType
DR = mybir.MatmulPerfMode.DoubleRow
F8NP = ml_dtypes.float8_e4m3
BF16NP = ml_dtypes.bfloat16

N_CORES = 8
B, S, D, DFF = 16, 1024, 768, 3072
BL = B // N_CORES          # batches per core
DC = D // 128              # 6  d-chunks
FC = DFF // 128            # 24 f-chunks
SC = S // 128              # 8  s-chunks
LN_EPS = 1e-5
SCALE = 1.0 / float(np.sqrt(np.float32(D)))
EH = D // 2                # 384: e-half for psum tiles
WS = 64.0                  # fp8 weight pre-scale (host); undone at evac
WS_INV = 1.0 / WS
WSM = 2048.0               # fp8 pre-scale for M = wq@wk^T (entries ~0.01)
WST = 16.0                 # fp8 scale for tmp = src@M
WST_WSM = WST / WSM        # tmp evac scale
SCALE_T = SCALE / WST      # scores evac scale


def _patched_drain_and_barrier(self, tick_clock, wait_clock):
    # This walrus build allows only one sync-wait per CTRL instruction;
    # split the tail-drain's global-clock waits across single-wait nops.
    nc = self.nc
    sink = nc.sync.nop()
    wait_clock.add_sem_waits(sink.ins, ScopedClock({None: tick_clock.global_clock}))
    si = sink.ins.sync_info
    waits = list(si.on_wait) if si is not None else []
    if si is not None:
        sink.ins.sync_info = bass_rust.SyncInfo(
            on_wait=waits[:1], on_update=list(si.on_update)
        )
    for w in waits[1:]:
        n = nc.sync.nop()
        n.ins.sync_info = bass_rust.SyncInfo(on_wait=[w], on_update=[])
    nc.sync.drain()
    nc.all_engine_barrier()
    popped = nc._tile_sem_poison_stack.pop()
    assert popped is self._sem_poison
    nc.clear_and_free_semaphores(list(self.sems.allocated().values()))
    nc.all_engine_barrier()


def _split_multi_waits(nc):
    # Same walrus limitation for every instruction class: hoist all but one
    # sync-wait onto same-engine NoOps inserted right before the offender.
    ctr = 0
    for f in nc.m.functions:
        for bb in f.blocks:
            out = []
            changed = False
            for inst in bb.instructions:
                si = inst.sync_info
                waits = list(si.on_wait) if si is not None else []
                if len(waits) > 1:
                    for w in waits[:-1]:
                        ctr += 1
                        n = bass_rust.InstNoOp(name=f"I-ws{ctr}", ins=[], outs=[])
                        n.engine = inst.engine
                        n.sync_info = bass_rust.SyncInfo(on_wait=[w], on_update=[])
                        out.append(n)
                    inst.sync_info = bass_rust.SyncInfo(
                        on_wait=[waits[-1]], on_update=list(si.on_update)
                    )
                    changed = True
                out.append(inst)
            if changed:
                bb.instructions = out


def _bcast_ap(vec_ap, parts=128):
    # view a [n] dram vector as [parts, n] with partition step 0
    return bass.AP(tensor=vec_ap.tensor, offset=vec_ap.offset,
                   ap=[[0, parts]] + list(vec_ap.ap))


def build_program():
    tile.TileContext._drain_and_barrier = _patched_drain_and_barrier

    nc = bass.Bass("TRN2", target_bir_lowering=False, debug=False)

    src_bf_d = nc.dram_tensor("src_bf", [BL, S, D], BF16, kind="ExternalInput").ap()
    srcT8_d = nc.dram_tensor("srcT8", [BL, 128, DC, S], F8, kind="ExternalInput").ap()
    srcTbf_d = nc.dram_tensor("srcT_bf", [BL, 128, DC, S], BF16, kind="ExternalInput").ap()
    gmT_d = nc.dram_tensor("gmT", [BL, S, S], FP16, kind="ExternalInput").ap()
    m8_d = nc.dram_tensor("m8", [128, DC, D], F8, kind="ExternalInput").ap()
    wv_bf_d = nc.dram_tensor("wv_bf", [128, DC, D], BF16, kind="ExternalInput").ap()
    w1bf_d = nc.dram_tensor("w1_bf", [128, DC, DFF], BF16, kind="ExternalInput").ap()
    w28_d = nc.dram_tensor("w28", [128, FC, D], F8, kind="ExternalInput").ap()
    bv = nc.dram_tensor("bv", [D], BF16, kind="ExternalInput").ap()
    b1 = nc.dram_tensor("b1", [128, FC], FP32, kind="ExternalInput").ap()
    ln1_w = nc.dram_tensor("ln1_w", [D], BF16, kind="ExternalInput").ap()
    ln1b2 = nc.dram_tensor("ln1b2", [D], BF16, kind="ExternalInput").ap()
    ln2_w = nc.dram_tensor("ln2_w", [D], BF16, kind="ExternalInput").ap()
    ln2_b = nc.dram_tensor("ln2_b", [D], BF16, kind="ExternalInput").ap()
    out = nc.dram_tensor("out", [BL, S, D], FP16, kind="ExternalOutput").ap()

    from contextlib import ExitStack

    with tile.TileContext(nc) as tc:
        root = ExitStack()
        with root:
            consts = root.enter_context(tc.tile_pool(name="consts", bufs=1))
            wqkv = root.enter_context(tc.tile_pool(name="wqkv", bufs=1))
            wffn = root.enter_context(tc.tile_pool(name="wffn", bufs=1))
            srcp = root.enter_context(tc.tile_pool(name="srcp", bufs=1))
            ps_a = root.enter_context(tc.tile_pool(name="ps_a", bufs=1, space="PSUM"))
            ps_b = root.enter_context(tc.tile_pool(name="ps_b", bufs=1, space="PSUM"))

            # ---- persistent weights (host-prepped); m8 split by e-half so
            # the first matmul groups gate on a quarter-sized load ----
            m8_a = wqkv.tile([128, DC, EH], F8)
            m8_b = wqkv.tile([128, DC, EH], F8)
            wv_t = wqkv.tile([128, DC, D], BF16)
            w1_t = wffn.tile([128, DC, DFF], BF16)
            w2_t = wffn.tile([128, FC, D], F8)

            # ---- constants (small, on scalar queue) ----
            b1_t = consts.tile([128, FC], FP32)
            nc.scalar.dma_start(b1_t[:], b1)
            eps_t = consts.tile([128, 1], FP32)
            nc.vector.memset(eps_t[:], LN_EPS)
            bv_b = consts.tile([128, D], BF16)
            ln1_wb = consts.tile([128, D], BF16)
            ln1b2_b = consts.tile([128, D], BF16)   # ln1_b + b2 (host-folded)
            ln2_wb = consts.tile([128, D], BF16)
            ln2_bb = consts.tile([128, D], BF16)

            # ---- startup loads: the first matmuls need m8 + srcT8, split
            # across queues and by half so early groups gate on less data
            nc.scalar.dma_start(m8_a[:], m8_d[:, :, 0:EH])
            nc.sync.dma_start(m8_b[:], m8_d[:, :, EH:D])

            src_bfs = [None] * BL
            srcT8s = [None] * BL

            def open_src(b):
                # srcT8 split by s-half: tmp group (ec, sh) reads only half sh
                sT8a = srcp.tile([128, DC, 512], F8, tag="srcT8a", bufs=2)
                sT8b = srcp.tile([128, DC, 512], F8, tag="srcT8b", bufs=2)
                if b == 0:
                    nc.sync.dma_start(sT8a[:], srcT8_d[b][:, :, 0:512])
                    nc.gpsimd.dma_start(sT8b[:], srcT8_d[b][:, :, 512:S])
                    nc.gpsimd.dma_start(wv_t[:], wv_bf_d)
                else:
                    nc.sync.dma_start(sT8a[:], srcT8_d[b][:, :, 0:512])
                    nc.sync.dma_start(sT8b[:], srcT8_d[b][:, :, 512:S])
                srcT8s[b] = (sT8a, sT8b)
                sbf = srcp.tile([128, SC, D], BF16, tag="srcbf", bufs=1)
                nc.gpsimd.dma_start(
                    sbf[:], src_bf_d[b].rearrange("(sc p) e -> p sc e", p=128))
                src_bfs[b] = sbf

            open_src(0)
            nc.sync.dma_start(bv_b[:], _bcast_ap(bv))
            nc.sync.dma_start(ln1_wb[:], _bcast_ap(ln1_w))
            nc.sync.dma_start(ln1b2_b[:], _bcast_ap(ln1b2))
            nc.sync.dma_start(ln2_wb[:], _bcast_ap(ln2_w))
            nc.sync.dma_start(ln2_bb[:], _bcast_ap(ln2_b))

            for b in range(BL):
                src_bf = src_bfs[b]
                sT8a, sT8b = srcT8s[b]
                es_late = ExitStack()
                late = es_late.enter_context(
                    tc.tile_pool(name=f"late{b}", bufs=1))
                es_proj = ExitStack()
                proj = es_proj.enter_context(
                    tc.tile_pool(name=f"proj{b}", bufs=1))
                srcT_bf = proj.tile([128, DC, S], BF16, tag="srcTbf")
                # b0 on scalar (early); later batches on sync so the WAR
                # wait cannot clog the scalar queue ahead of evacuations
                (nc.scalar if b == 0 else nc.sync).dma_start(
                    srcT_bf[:], srcTbf_d[b])

                # -------- tmp = src@M projection (fp8 DR) --------
                # scores = src M src^T (+ host-folded bias terms in gmT)
                tmpT8 = proj.tile([128, DC, S], F8, tag="tmpT8")
                v_ext = proj.tile([128, SC, 2 * EH + 1], BF16, tag="vext")
                nc.vector.memset(v_ext[:, :, 2 * EH:2 * EH + 1], 1.0)
                for ec in range(DC):
                    m8h = m8_a if ec < 3 else m8_b
                    eo = (ec % 3) * 128
                    ps = ps_a.tile([128, 2, 512], FP32, tag="ps_mm", bufs=2)
                    for sh in range(2):
                        s8h = sT8a if sh == 0 else sT8b
                        for t in range(3):
                            nc.tensor.matmul(
                                ps[:, sh, :],
                                m8h[:, 2 * t:2 * t + 2, eo:eo + 128],
                                s8h[:, 2 * t:2 * t + 2, :],
                                start=(t == 0), stop=(t == 2), perf_mode=DR,
                            )
                    nc.scalar.activation(
                        tmpT8[:, ec, :], ps[:, :, :],
                        AF.Identity, scale=WST_WSM,
                    )

                # ---------- scores^T + softmax (per k-block) ----------
                PT_bf = late.tile([128, SC, S], BF16, tag="PT")
                xn_big = late.tile([128, SC, D], BF16, tag="xn_big")
                yT_lo = late.tile([128, DC, 512], BF16, tag="yTl")
                yT_hi = late.tile([128, DC, 512], BF16, tag="yTh")
                with tc.tile_pool(name=f"attn{b}", bufs=2) as at_pool:
                    for kb in range(SC):
                        g_t = at_pool.tile([128, S], FP16, tag="g", bufs=3)
                        nc.gpsimd.dma_start(
                            g_t[:], gmT_d[b, kb * 128:(kb + 1) * 128, :])
                        # FFN weights stream in mid-attention (once)
                        if b == 0 and kb == 1:
                            nc.sync.dma_start(w1_t[:], w1bf_d)
                        elif b == 0 and kb == 3:
                            nc.scalar.dma_start(w2_t[:], w28_d)
                        ps = ps_a.tile([128, 2, 512], FP32, tag="ps_mm", bufs=2)
                        s8h = sT8a if kb < 4 else sT8b
                        ko = (kb % 4) * 128
                        for qh in range(2):
                            for t in range(3):
                                nc.tensor.matmul(
                                    ps[:, qh, :],
                                    s8h[:, 2 * t:2 * t + 2, ko:ko + 128],
                                    tmpT8[:, 2 * t:2 * t + 2, qh * 512:(qh + 1) * 512],
                                    start=(t == 0), stop=(t == 2), perf_mode=DR,
                                )
                        expin = at_pool.tile([128, S], FP32, tag="expin", bufs=2)
                        nc.vector.scalar_tensor_tensor(
                            out=expin[:], in0=ps[:, :, :], scalar=SCALE_T,
                            in1=g_t[:], op0=OP.mult, op1=OP.subtract,
                        )
                        nc.scalar.activation(PT_bf[:, kb, :], expin[:], AF.Exp)

                    # ---------- V projection (bf16), after scores so the
                    # srcT_bf prefetch has the whole tmp+scores span --------
                    for sc in range(SC):
                        ps = ps_b.tile([128, 2, 512], FP32, tag="ps_pv", bufs=2)
                        for eh in range(2):
                            for dc in range(DC):
                                nc.tensor.matmul(
                                    ps[:, eh, 0:EH],
                                    srcT_bf[:, dc, sc * 128:(sc + 1) * 128],
                                    wv_t[:, dc, eh * EH:(eh + 1) * EH],
                                    start=(dc == 0), stop=(dc == DC - 1),
                                )
                        nc.vector.scalar_tensor_tensor(
                            out=v_ext[:, sc, 0:2 * EH],
                            in0=ps[:, :, 0:EH], scalar=1.0,
                            in1=bv_b[:],
                            op0=OP.mult, op1=OP.add,
                        )

                    # ---------- PV + LN1 (per q-block, apply 1-deferred) ----
                    defer = [None]

                    def ln1_apply(qc, resid, mv, rstd):
                        nmr = at_pool.tile([128, 1], FP32, tag="nmr", bufs=2)
                        nc.gpsimd.tensor_scalar(
                            out=nmr[:], in0=mv[:, 0:1], scalar1=rstd[:],
                            scalar2=-1.0, op0=OP.mult, op1=OP.mult)
                        # y lands in xn_big (fixed up to x+b2 in-place later)
                        nc.scalar.activation(
                            xn_big[:, qc, :], resid[:], AF.Identity,
                            bias=nmr[:], scale=rstd[:])
                        # y -> yT (strided slice) via one XBAR DMA transpose
                        yT = yT_lo if qc < 4 else yT_hi
                        nc.sync.dma_start_transpose(
                            yT[:, 0:DC, (qc % 4) * 128:(qc % 4 + 1) * 128],
                            xn_big[:, qc, :])

                    for qc in range(SC):
                        ps = ps_b.tile([128, 2, 512], FP32, tag="ps_pv", bufs=2)
                        for kc in range(SC):
                            nc.tensor.matmul(
                                ps[:, 0, 0:EH],
                                PT_bf[:, kc, qc * 128:(qc + 1) * 128],
                                v_ext[:, kc, 0:EH],
                                start=(kc == 0), stop=(kc == SC - 1),
                            )
                        for kc in range(SC):
                            nc.tensor.matmul(
                                ps[:, 1, 0:EH + 1],
                                PT_bf[:, kc, qc * 128:(qc + 1) * 128],
                                v_ext[:, kc, EH:2 * EH + 1],
                                start=(kc == 0), stop=(kc == SC - 1),
                            )
                        zinv = at_pool.tile([128, 1], FP32, tag="zi", bufs=3)
                        nc.vector.reciprocal(zinv[:], ps[:, 1, EH:EH + 1])
                        resid = at_pool.tile([128, D], FP32, tag="resid", bufs=3)
                        nc.vector.scalar_tensor_tensor(
                            out=resid[:], in0=ps[:, :, 0:EH], scalar=zinv[:],
                            in1=src_bf[:, qc, :], op0=OP.mult, op1=OP.add)
                        stats = at_pool.tile([128, 2, 6], FP32, tag="st")
                        nc.vector.bn_stats(stats[:, 0, :], resid[:, 0:512])
                        nc.vector.bn_stats(stats[:, 1, :], resid[:, 512:768])
                        mv = at_pool.tile([128, 2], FP32, tag="mv")
                        nc.vector.bn_aggr(mv[:], stats[:])
                        rstd = at_pool.tile([128, 1], FP32, tag="rstd", bufs=2)
                        nc.scalar.activation(
                            rstd[:], mv[:, 1:2], AF.Ln, bias=eps_t[:])
                        nc.scalar.activation(rstd[:], rstd[:], AF.Exp, scale=-0.5)
                        if defer[0] is not None:
                            ln1_apply(*defer[0])
                        defer[0] = (qc, resid, mv, rstd)
                    ln1_apply(*defer[0])

                    # in-place xn fixup: xn = y*ln1_w + (ln1_b+b2); runs on
                    # DVE/GpSimd during FFN1 when both queues are idle
                    for qc in range(SC):
                        nc.vector.tensor_tensor(
                            out=xn_big[:, qc, :], in0=xn_big[:, qc, :],
                            in1=ln1_wb[:], op=OP.mult)
                        nc.gpsimd.tensor_tensor(
                            out=xn_big[:, qc, :], in0=xn_big[:, qc, :],
                            in1=ln1b2_b[:], op=OP.add)

                es_proj.close()

                # ---------- FFN1 (bf16, qh-outer so qh0 covers LN1 tail) ----
                with tc.tile_pool(name=f"ffn{b}", bufs=1) as ffn_pool, \
                     tc.tile_pool(name=f"ffn2{b}", bufs=2) as f2_pool:
                    # prefetch next batch's src while FFN runs
                    if b + 1 < BL:
                        open_src(b + 1)
                    hT8 = ffn_pool.tile([128, FC, S], F8, tag="hT8")
                    for qh in range(2):
                        yT = yT_lo if qh == 0 else yT_hi
                        for f2 in range(FC // 2):
                            ps = ps_a.tile([128, 2, 512], FP32, tag="ps_mm",
                                           bufs=2)
                            for fh in range(2):
                                fc = 2 * f2 + fh
                                for dc in range(DC):
                                    nc.tensor.matmul(
                                        ps[:, fh, :],
                                        w1_t[:, dc, fc * 128:(fc + 1) * 128],
                                        yT[:, dc, :],
                                        start=(dc == 0), stop=(dc == DC - 1),
                                    )
                            for fh in range(2):
                                fc = 2 * f2 + fh
                                nc.scalar.activation(
                                    hT8[:, fc, qh * 512:(qh + 1) * 512],
                                    ps[:, fh, :],
                                    AF.Gelu, bias=b1_t[:, fc:fc + 1],
                                )
                    # ---------- FFN2 + LN2 (per q-block) ----------
                    defer2 = [None]

                    def ln2_apply(sc, ypre, mv, rstd):
                        nmr = f2_pool.tile([128, 1], FP32, tag="nmr2")
                        nc.vector.tensor_scalar(
                            out=nmr[:], in0=mv[:, 0:1], scalar1=rstd[:],
                            scalar2=-1.0, op0=OP.mult, op1=OP.mult)
                        y2 = f2_pool.tile([128, D], FP32, tag="y2")
                        nc.scalar.activation(
                            y2[:], ypre[:], AF.Identity, bias=nmr[:],
                            scale=rstd[:])
                        ow = f2_pool.tile([128, D], FP32, tag="ow")
                        nc.vector.tensor_tensor(
                            out=ow[:], in0=y2[:], in1=ln2_wb[:], op=OP.mult)
                        o_t = f2_pool.tile([128, D], FP16, tag="o")
                        nc.gpsimd.tensor_tensor(
                            out=o_t[:], in0=ow[:], in1=ln2_bb[:], op=OP.add)
                        eng = nc.sync if sc % 2 == 0 else nc.gpsimd
                        eng.dma_start(
                            out[b, sc * 128:(sc + 1) * 128, :], o_t[:])

                    for sc in range(SC):
                        ps = ps_b.tile([128, 2, 512], FP32, tag="ps_pv", bufs=2)
                        for eh in range(2):
                            for tf in range(FC // 2):
                                nc.tensor.matmul(
                                    ps[:, eh, 0:EH],
                                    hT8[:, 2 * tf:2 * tf + 2, sc * 128:(sc + 1) * 128],
                                    w2_t[:, 2 * tf:2 * tf + 2, eh * EH:(eh + 1) * EH],
                                    start=(tf == 0), stop=(tf == FC // 2 - 1),
                                    perf_mode=DR,
                                )
                        ypre = f2_pool.tile([128, D], FP32, tag="ypre", bufs=3)
                        nc.vector.scalar_tensor_tensor(
                            out=ypre[:], in0=ps[:, :, 0:EH], scalar=WS_INV,
                            in1=xn_big[:, sc, :], op0=OP.mult, op1=OP.add)
                        stats = f2_pool.tile([128, 2, 6], FP32, tag="st2")
                        nc.vector.bn_stats(stats[:, 0, :], ypre[:, 0:512])
                        nc.vector.bn_stats(stats[:, 1, :], ypre[:, 512:768])
                        mv = f2_pool.tile([128, 2], FP32, tag="mv2")
                        nc.vector.bn_aggr(mv[:], stats[:])
                        rstd2 = f2_pool.tile([128, 1], FP32, tag="rstd2")
                        nc.scalar.activation(
                            rstd2[:], mv[:, 1:2], AF.Ln, bias=eps_t[:])
                        nc.scalar.activation(
                            rstd2[:], rstd2[:], AF.Exp, scale=-0.5)
                        if defer2[0] is not None:
                            ln2_apply(*defer2[0])
                        defer2[0] = (sc, ypre, mv, rstd2)
                    ln2_apply(*defer2[0])
                es_late.close()

    _split_multi_waits(nc)
    return nc


_NC_CACHE = None


def kernel(**inputs):
    global _NC_CACHE
    if _NC_CACHE is None:
        _NC_CACHE = build_program()
    nc = _NC_CACHE

    f32 = lambda k: np.asarray(inputs[k], dtype=np.float32)

    # scores = q@k^T = src M src^T + u[q] + w[k] + c with M = wq wk^T,
    # u = src@(wq bk), w = src@(wk bq), c = bq.bk  (bias terms -> gmT)
    Mqk = f32("wq") @ f32("wk").T
    m8 = np.ascontiguousarray(
        (Mqk * WSM).reshape(DC, 128, D).transpose(1, 0, 2)).astype(F8NP)
    wv_bf = np.ascontiguousarray(
        f32("wv").reshape(DC, 128, D).transpose(1, 0, 2)).astype(BF16NP)
    # fold ln1 affine into FFN1 (exact): gelu((y*w+b)@w1 + b1)
    #   = gelu(y@(diag(w)w1) + (b1 + b@w1))
    w1f = f32("w1") * f32("ln1_w")[:, None]
    b1f = f32("b1") + f32("ln1_b") @ f32("w1")
    w1_bf = np.ascontiguousarray(
        w1f.reshape(DC, 128, DFF).transpose(1, 0, 2)).astype(BF16NP)
    w28 = np.ascontiguousarray(
        (f32("w2") * WS).reshape(FC, 128, D).transpose(1, 0, 2)).astype(F8NP)

    shared = {
        "m8": m8, "wv_bf": wv_bf, "w1_bf": w1_bf, "w28": w28,
        "bv": f32("bv").astype(BF16NP),
        "b1": np.ascontiguousarray(b1f.reshape(FC, 128).T),
        "ln1_w": f32("ln1_w").astype(BF16NP),
        "ln1b2": (f32("ln1_b") + f32("b2")).astype(BF16NP),
        "ln2_w": f32("ln2_w").astype(BF16NP),
        "ln2_b": f32("ln2_b").astype(BF16NP),
    }
    src = np.asarray(inputs["src"], dtype=np.float32)
    gum = np.asarray(inputs["gumbel_u"], dtype=np.float32)
    # gumbel + score bias terms: expin = (srcMsrc^T)*scale - gm' with
    # gm'[q,k] = ln(-ln u) - scale*(u[q] + w[k] + c); then transpose
    uq = src @ (f32("wq") @ f32("bk"))            # [B,S]
    wk_ = src @ (f32("wk") @ f32("bq"))           # [B,S]
    cc = float(f32("bq") @ f32("bk"))
    gm = np.log(-np.log(gum))
    gm -= SCALE * (uq[:, :, None] + wk_[:, None, :] + cc)
    gmT = gm.transpose(0, 2, 1)

    in_maps = []
    for c in range(N_CORES):
        m = dict(shared)
        sc_ = src[c * BL:(c + 1) * BL]
        m["src_bf"] = np.ascontiguousarray(sc_.astype(BF16NP))
        srcT = np.ascontiguousarray(
            sc_.reshape(BL, S, DC, 128).transpose(0, 3, 2, 1))
        m["srcT8"] = srcT.astype(F8NP)
        m["srcT_bf"] = srcT.astype(BF16NP)
        m["gmT"] = np.ascontiguousarray(
            gmT[c * BL:(c + 1) * BL]).astype(np.float16)
        in_maps.append(m)

    res = run_bass_kernel_spmd(nc, in_maps, core_ids=list(range(N_CORES)))
    return np.concatenate(
        [res.results[c]["out"].astype(np.float32) for c in range(N_CORES)],
        axis=0)


# revision 36
# speedup vs baseline: 1.0172x; 1.0172x over previous
"""Trainium2 Bass kernel for a single-head transformer encoder layer with
gumbel-softmax attention.

Reference computation (per batch):
    q,k,v = src@wq+bq, src@wk+bk, src@wv+bv
    attn  = softmax(q@k.T/sqrt(D) + (-log(-log(u))))
    x     = LN1(src + attn@v)
    out   = LN2(x + gelu(x@w1+b1)@w2 + b2)

Sharding: pure data-parallel over batch. B=16 over 8 cores -> 2 batches/core.

v5 design:
  - scores computed TRANSPOSED (S^T[k,q] = k @ q^T per 128-k block) so the
    post-exp matrix is already in the [k-part, q-free] stationary layout the
    PV matmul wants -> no P transposes. Gumbel noise ln(-ln u) is
    host-precomputed (fp16, transposed) -> no ACT Ln passes.
  - softmax denominator z rides a ones-column appended to v (v_ext[:,:,768])
    so PV emits [q, 384|384+1] and z lands per-q-partition.
  - QK projection fp8 DoubleRow; FFN2 fp8 DoubleRow; V/PV/FFN1 bf16.
  - ln1_w/ln1_b folded into w1/b1 on host (exact): FFN1 consumes the raw
    LN1 output y; x=y*w+b (+b2, host-folded) is only materialized for the
    LN2 residual, on GpSimd, off the critical path.
  - the structural transpose (y [q,e] -> yT [e,q] for FFN1) is one XBAR DMA
    transpose per q-block writing the strided yT slice directly.
  - LN stats come from accum_out side-channels (residual stt emits sum(x),
    one ACT Square pass emits sum(x^2)) -> no bn_stats chains; the LN apply
    for block qc-1 is emitted during block qc so the ACT-computed rstd is
    never waited on in-queue.
  - PSUM tiles are [128,2,512] pairs so most evacuations cover two
    accumulation groups in one instruction.
  - ACT table sets phase-separated: attention+LN use natural_log_exp set
    (Square/Copy/Identity are in every set), FFN1 gelu evacs batch together.
"""

import numpy as np
import ml_dtypes

import bass_rust
import concourse.bass as bass
import concourse.tile as tile
from concourse import mybir
from concourse.vector_clock import ScopedClock
from concourse.bass_utils import run_bass_kernel_spmd

FP32 = mybir.dt.float32
FP16 = mybir.dt.float16
BF16 = mybir.dt.bfloat16
F8 = mybir.dt.float8e4
AF = mybir.ActivationFunctionType
OP = mybir.AluOpType
DR = mybir.MatmulPerfMode.DoubleRow
F8NP = ml_dtypes.float8_e4m3
BF16NP = ml_dtypes.bfloat16

N_CORES = 8
B, S, D, DFF = 16, 1024, 768, 3072
BL = B // N_CORES          # batches per core
DC = D // 128              # 6  d-chunks
FC = DFF // 128            # 24 f-chunks
SC = S // 128              # 8  s-chunks
LN_EPS = 1e-5
SCALE = 1.0 / float(np.sqrt(np.float32(D)))
EH = D // 2                # 384: e-half for psum tiles
WS = 64.0                  # fp8 weight pre-scale (host); undone at evac
WS_INV = 1.0 / WS
WSM = 2048.0               # fp8 pre-scale for M = wq@wk^T (entries ~0.01)
WST = 16.0                 # fp8 scale for tmp = src@M
WST_WSM = WST / WSM        # tmp evac scale
SCALE_T = SCALE / WST      # scores evac scale


def _patched_drain_and_barrier(self, tick_clock, wait_clock):
    # This walrus build allows only one sync-wait per CTRL instruction;
    # split the tail-drain's global-clock waits across single-wait nops.
    nc = self.nc
    sink = nc.sync.nop()
    wait_clock.add_sem_waits(sink.ins, ScopedClock({None: tick_clock.global_clock}))
    si = sink.ins.sync_info
    waits = list(si.on_wait) if si is not None else []
    if si is not None:
        sink.ins.sync_info = bass_rust.SyncInfo(
            on_wait=waits[:1], on_update=list(si.on_update)
        )
    for w in waits[1:]:
        n = nc.sync.nop()
        n.ins.sync_info = bass_rust.SyncInfo(on_wait=[w], on_update=[])
    nc.sync.drain()
    nc.all_engine_barrier()
    popped = nc._tile_sem_poison_stack.pop()
    assert popped is self._sem_poison
    nc.clear_and_free_semaphores(list(self.sems.allocated().values()))
    nc.all_engine_barrier()


def _split_multi_waits(nc):
    # Same walrus limitation for every instruction class: hoist all but one
    # sync-wait onto same-engine NoOps inserted right before the offender.
    ctr = 0
    for f in nc.m.functions:
        for bb in f.blocks:
            out = []
            changed = False
            for inst in bb.instructions:
                si = inst.sync_info
                waits = list(si.on_wait) if si is not None else []
                if len(waits) > 1:
                    for w in waits[:-1]:
                        ctr += 1
                        n = bass_rust.InstNoOp(name=f"I-ws{ctr}", ins=[], outs=[])
                        n.engine = inst.engine
                        n.sync_info = bass_rust.SyncInfo(on_wait=[w], on_update=[])
                        out.append(n)
                    inst.sync_info = bass_rust.SyncInfo(
                        on_wait=[waits[-1]], on_update=list(si.on_update)
                    )
                    changed = True
                out.append(inst)
            if changed:
                bb.instructions = out


def _bcast_ap(vec_ap, parts=128):
    # view a [n] dram vector as [parts, n] with partition step 0
    return bass.AP(tensor=vec_ap.tensor, offset=vec_ap.offset,
                   ap=[[0, parts]] + list(vec_ap.ap))


def build_program():
    tile.TileContext._drain_and_barrier = _patched_drain_and_barrier

    nc = bass.Bass("TRN2", target_bir_lowering=False, debug=False)

    src_bf_d = nc.dram_tensor("src_bf", [BL, S, D], BF16, kind="ExternalInput").ap()
    srcT8_d = nc.dram_tensor("srcT8", [BL, 128, DC, S], F8, kind="ExternalInput").ap()
    srcTbf_d = nc.dram_tensor("srcT_bf", [BL, 128, DC, S], BF16, kind="ExternalInput").ap()
    gmT_d = nc.dram_tensor("gmT", [BL, S, S], FP16, kind="ExternalInput").ap()
    m8_d = nc.dram_tensor("m8", [128, DC, D], F8, kind="ExternalInput").ap()
    wv_bf_d = nc.dram_tensor("wv_bf", [128, DC, D], BF16, kind="ExternalInput").ap()
    w1bf_d = nc.dram_tensor("w1_bf", [128, DC, DFF], BF16, kind="ExternalInput").ap()
    w28_d = nc.dram_tensor("w28", [128, FC, D], F8, kind="ExternalInput").ap()
    bv = nc.dram_tensor("bv", [D], BF16, kind="ExternalInput").ap()
    b1 = nc.dram_tensor("b1", [128, FC], FP32, kind="ExternalInput").ap()
    ln1_w = nc.dram_tensor("ln1_w", [D], BF16, kind="ExternalInput").ap()
    ln1b2 = nc.dram_tensor("ln1b2", [D], BF16, kind="ExternalInput").ap()
    ln2_w = nc.dram_tensor("ln2_w", [D], BF16, kind="ExternalInput").ap()
    ln2_b = nc.dram_tensor("ln2_b", [D], BF16, kind="ExternalInput").ap()
    out = nc.dram_tensor("out", [BL, S, D], FP16, kind="ExternalOutput").ap()

    from contextlib import ExitStack

    with tile.TileContext(nc) as tc:
        root = ExitStack()
        with root:
            consts = root.enter_context(tc.tile_pool(name="consts", bufs=1))
            wqkv = root.enter_context(tc.tile_pool(name="wqkv", bufs=1))
            wffn = root.enter_context(tc.tile_pool(name="wffn", bufs=1))
            srcp = root.enter_context(tc.tile_pool(name="srcp", bufs=1))
            ps_a = root.enter_context(tc.tile_pool(name="ps_a", bufs=1, space="PSUM"))
            ps_b = root.enter_context(tc.tile_pool(name="ps_b", bufs=1, space="PSUM"))

            # ---- persistent weights (host-prepped) ----
            m8_t = wqkv.tile([128, DC, D], F8)
            wv_t = wqkv.tile([128, DC, D], BF16)
            w1_t = wffn.tile([128, DC, DFF], BF16)
            w2_t = wffn.tile([128, FC, D], F8)

            # ---- constants (small, on scalar queue) ----
            b1_t = consts.tile([128, FC], FP32)
            nc.scalar.dma_start(b1_t[:], b1)
            eps_t = consts.tile([128, 1], FP32)
            nc.vector.memset(eps_t[:], LN_EPS)
            bv_b = consts.tile([128, D], BF16)
            ln1_wb = consts.tile([128, D], BF16)
            ln1b2_b = consts.tile([128, D], BF16)   # ln1_b + b2 (host-folded)
            ln2_wb = consts.tile([128, D], BF16)
            ln2_bb = consts.tile([128, D], BF16)

            # ---- startup loads: the first matmuls need m8 + srcT8, so both
            # are split in halves across queues to halve the load latency
            nc.scalar.dma_start(m8_t[:, 0:3, :], m8_d[:, 0:3, :])
            nc.sync.dma_start(m8_t[:, 3:6, :], m8_d[:, 3:6, :])

            src_bfs = [None] * BL
            srcT8s = [None] * BL

            def open_src(b):
                sT8 = srcp.tile([128, DC, S], F8, tag="srcT8", bufs=2)
                if b == 0:
                    nc.sync.dma_start(sT8[:, 0:3, :], srcT8_d[b][:, 0:3, :])
                    nc.gpsimd.dma_start(sT8[:, 3:6, :], srcT8_d[b][:, 3:6, :])
                    nc.gpsimd.dma_start(wv_t[:], wv_bf_d)
                else:
                    nc.sync.dma_start(sT8[:], srcT8_d[b])
                srcT8s[b] = sT8
                sbf = srcp.tile([128, SC, D], BF16, tag="srcbf", bufs=1)
                nc.gpsimd.dma_start(
                    sbf[:], src_bf_d[b].rearrange("(sc p) e -> p sc e", p=128))
                src_bfs[b] = sbf

            open_src(0)

            # tmp = src@M projection, root-level so batch b+1's projection
            # can run inside batch b's FFN2 phase (cross-batch pipelining)
            tmpT8s = [None] * BL

            def emit_tmp(b):
                srcT8 = srcT8s[b]
                t8 = srcp.tile([128, DC, S], F8, tag="tmpT8", bufs=1)
                for ec in range(DC):
                    ps = ps_a.tile([128, 2, 512], FP32, tag="ps_mm", bufs=2)
                    for sh in range(2):
                        for t in range(3):
                            nc.tensor.matmul(
                                ps[:, sh, :],
                                m8_t[:, 2 * t:2 * t + 2, ec * 128:(ec + 1) * 128],
                                srcT8[:, 2 * t:2 * t + 2, sh * 512:(sh + 1) * 512],
                                start=(t == 0), stop=(t == 2), perf_mode=DR,
                            )
                    nc.scalar.activation(
                        t8[:, ec, :], ps[:, :, :],
                        AF.Identity, scale=WST_WSM,
                    )
                tmpT8s[b] = t8

            emit_tmp(0)
            nc.sync.dma_start(bv_b[:], _bcast_ap(bv))
            nc.sync.dma_start(ln1_wb[:], _bcast_ap(ln1_w))
            nc.sync.dma_start(ln1b2_b[:], _bcast_ap(ln1b2))
            nc.sync.dma_start(ln2_wb[:], _bcast_ap(ln2_w))
            nc.sync.dma_start(ln2_bb[:], _bcast_ap(ln2_b))

            for b in range(BL):
                src_bf = src_bfs[b]
                srcT8 = srcT8s[b]
                es_late = ExitStack()
                late = es_late.enter_context(
                    tc.tile_pool(name=f"late{b}", bufs=1))
                es_proj = ExitStack()
                proj = es_proj.enter_context(
                    tc.tile_pool(name=f"proj{b}", bufs=1))
                srcT_bf = proj.tile([128, DC, S], BF16, tag="srcTbf")
                # b0 on scalar (early); later batches on sync so the WAR
                # wait cannot clog the scalar queue ahead of evacuations
                (nc.scalar if b == 0 else nc.sync).dma_start(
                    srcT_bf[:], srcTbf_d[b])

                # tmp projection already emitted (cross-batch pipelined)
                tmpT8 = tmpT8s[b]
                v_ext = proj.tile([128, SC, 2 * EH + 1], BF16, tag="vext")
                nc.vector.memset(v_ext[:, :, 2 * EH:2 * EH + 1], 1.0)

                # ---------- scores^T + softmax (per k-block) ----------
                PT_bf = late.tile([128, SC, S], BF16, tag="PT")
                xn_big = late.tile([128, SC, D], BF16, tag="xn_big")
                yT_lo = late.tile([128, DC, 512], BF16, tag="yTl")
                yT_hi = late.tile([128, DC, 512], BF16, tag="yTh")
                with tc.tile_pool(name=f"attn{b}", bufs=2) as at_pool:
                    for kb in range(SC):
                        g_t = at_pool.tile([128, S], FP16, tag="g", bufs=3)
                        nc.gpsimd.dma_start(
                            g_t[:], gmT_d[b, kb * 128:(kb + 1) * 128, :])
                        # FFN weights stream in mid-attention (once)
                        if b == 0 and kb == 1:
                            nc.sync.dma_start(w1_t[:], w1bf_d)
                        elif b == 0 and kb == 3:
                            nc.scalar.dma_start(w2_t[:], w28_d)
                        ps = ps_a.tile([128, 2, 512], FP32, tag="ps_mm", bufs=2)
                        for qh in range(2):
                            for t in range(3):
                                nc.tensor.matmul(
                                    ps[:, qh, :],
                                    srcT8[:, 2 * t:2 * t + 2, kb * 128:(kb + 1) * 128],
                                    tmpT8[:, 2 * t:2 * t + 2, qh * 512:(qh + 1) * 512],
                                    start=(t == 0), stop=(t == 2), perf_mode=DR,
                                )
                        expin = at_pool.tile([128, S], FP32, tag="expin", bufs=2)
                        nc.vector.scalar_tensor_tensor(
                            out=expin[:], in0=ps[:, :, :], scalar=SCALE_T,
                            in1=g_t[:], op0=OP.mult, op1=OP.subtract,
                        )
                        nc.scalar.activation(PT_bf[:, kb, :], expin[:], AF.Exp)

                    # ---------- V projection (bf16), after scores so the
                    # srcT_bf prefetch has the whole tmp+scores span --------
                    for sc in range(SC):
                        ps = ps_b.tile([128, 2, 512], FP32, tag="ps_pv", bufs=2)
                        for eh in range(2):
                            for dc in range(DC):
                                nc.tensor.matmul(
                                    ps[:, eh, 0:EH],
                                    srcT_bf[:, dc, sc * 128:(sc + 1) * 128],
                                    wv_t[:, dc, eh * EH:(eh + 1) * EH],
                                    start=(dc == 0), stop=(dc == DC - 1),
                                )
                        nc.vector.scalar_tensor_tensor(
                            out=v_ext[:, sc, 0:2 * EH],
                            in0=ps[:, :, 0:EH], scalar=1.0,
                            in1=bv_b[:],
                            op0=OP.mult, op1=OP.add,
                        )

                    # ---------- PV + LN1 (per q-block, apply 1-deferred) ----
                    defer = [None]

                    def ln1_apply(qc, resid, mv, rstd):
                        nmr = at_pool.tile([128, 1], FP32, tag="nmr", bufs=2)
                        nc.vector.tensor_scalar(
                            out=nmr[:], in0=mv[:, 0:1], scalar1=rstd[:],
                            scalar2=-1.0, op0=OP.mult, op1=OP.mult)
                        # y lands in xn_big (fixed up to x+b2 in-place later)
                        nc.scalar.activation(
                            xn_big[:, qc, :], resid[:], AF.Identity,
                            bias=nmr[:], scale=rstd[:])
                        # y -> yT (strided slice) via one XBAR DMA transpose
                        yT = yT_lo if qc < 4 else yT_hi
                        nc.sync.dma_start_transpose(
                            yT[:, 0:DC, (qc % 4) * 128:(qc % 4 + 1) * 128],
                            xn_big[:, qc, :])

                    for qc in range(SC):
                        ps = ps_b.tile([128, 2, 512], FP32, tag="ps_pv", bufs=2)
                        for kc in range(SC):
                            nc.tensor.matmul(
                                ps[:, 0, 0:EH],
                                PT_bf[:, kc, qc * 128:(qc + 1) * 128],
                                v_ext[:, kc, 0:EH],
                                start=(kc == 0), stop=(kc == SC - 1),
                            )
                        for kc in range(SC):
                            nc.tensor.matmul(
                                ps[:, 1, 0:EH + 1],
                                PT_bf[:, kc, qc * 128:(qc + 1) * 128],
                                v_ext[:, kc, EH:2 * EH + 1],
                                start=(kc == 0), stop=(kc == SC - 1),
                            )
                        zinv = at_pool.tile([128, 1], FP32, tag="zi", bufs=3)
                        nc.vector.reciprocal(zinv[:], ps[:, 1, EH:EH + 1])
                        resid = at_pool.tile([128, D], FP32, tag="resid", bufs=3)
                        nc.vector.scalar_tensor_tensor(
                            out=resid[:], in0=ps[:, :, 0:EH], scalar=zinv[:],
                            in1=src_bf[:, qc, :], op0=OP.mult, op1=OP.add)
                        stats = at_pool.tile([128, 2, 6], FP32, tag="st")
                        nc.vector.bn_stats(stats[:, 0, :], resid[:, 0:512])
                        nc.vector.bn_stats(stats[:, 1, :], resid[:, 512:768])
                        mv = at_pool.tile([128, 2], FP32, tag="mv")
                        nc.vector.bn_aggr(mv[:], stats[:])
                        rstd = at_pool.tile([128, 1], FP32, tag="rstd", bufs=2)
                        nc.scalar.activation(
                            rstd[:], mv[:, 1:2], AF.Ln, bias=eps_t[:])
                        nc.scalar.activation(rstd[:], rstd[:], AF.Exp, scale=-0.5)
                        if defer[0] is not None:
                            ln1_apply(*defer[0])
                        defer[0] = (qc, resid, mv, rstd)
                    ln1_apply(*defer[0])

                    # in-place xn fixup: xn = y*ln1_w + (ln1_b+b2); runs on
                    # DVE/GpSimd during FFN1 when both queues are idle
                    for qc in range(SC):
                        nc.vector.tensor_tensor(
                            out=xn_big[:, qc, :], in0=xn_big[:, qc, :],
                            in1=ln1_wb[:], op=OP.mult)
                        nc.gpsimd.tensor_tensor(
                            out=xn_big[:, qc, :], in0=xn_big[:, qc, :],
                            in1=ln1b2_b[:], op=OP.add)

                es_proj.close()

                # ---------- FFN1 (bf16, qh-outer so qh0 covers LN1 tail) ----
                with tc.tile_pool(name=f"ffn{b}", bufs=1) as ffn_pool, \
                     tc.tile_pool(name=f"ffn2{b}", bufs=2) as f2_pool:
                    # prefetch next batch's src while FFN runs
                    if b + 1 < BL:
                        open_src(b + 1)
                    hT8 = ffn_pool.tile([128, FC, S], F8, tag="hT8")
                    for qh in range(2):
                        yT = yT_lo if qh == 0 else yT_hi
                        for f2 in range(FC // 2):
                            ps = ps_a.tile([128, 2, 512], FP32, tag="ps_mm",
                                           bufs=2)
                            for fh in range(2):
                                fc = 2 * f2 + fh
                                for dc in range(DC):
                                    nc.tensor.matmul(
                                        ps[:, fh, :],
                                        w1_t[:, dc, fc * 128:(fc + 1) * 128],
                                        yT[:, dc, :],
                                        start=(dc == 0), stop=(dc == DC - 1),
                                    )
                            for fh in range(2):
                                fc = 2 * f2 + fh
                                nc.scalar.activation(
                                    hT8[:, fc, qh * 512:(qh + 1) * 512],
                                    ps[:, fh, :],
                                    AF.Gelu, bias=b1_t[:, fc:fc + 1],
                                )
                    # ---------- FFN2 + LN2 (per q-block) ----------
                    defer2 = [None]

                    def ln2_apply(sc, ypre, mv, rstd):
                        nmr = f2_pool.tile([128, 1], FP32, tag="nmr2")
                        nc.vector.tensor_scalar(
                            out=nmr[:], in0=mv[:, 0:1], scalar1=rstd[:],
                            scalar2=-1.0, op0=OP.mult, op1=OP.mult)
                        y2 = f2_pool.tile([128, D], FP32, tag="y2")
                        nc.scalar.activation(
                            y2[:], ypre[:], AF.Identity, bias=nmr[:],
                            scale=rstd[:])
                        ow = f2_pool.tile([128, D], FP32, tag="ow")
                        nc.vector.tensor_tensor(
                            out=ow[:], in0=y2[:], in1=ln2_wb[:], op=OP.mult)
                        o_t = f2_pool.tile([128, D], FP16, tag="o")
                        nc.gpsimd.tensor_tensor(
                            out=o_t[:], in0=ow[:], in1=ln2_bb[:], op=OP.add)
                        eng = nc.sync if sc % 2 == 0 else nc.gpsimd
                        eng.dma_start(
                            out[b, sc * 128:(sc + 1) * 128, :], o_t[:])

                    for sc in range(SC):
                        # batch b+1's tmp projection rides here, after its
                        # srcT8 prefetch (issued at FFN1 start) has landed
                        if sc == 4 and b + 1 < BL:
                            emit_tmp(b + 1)
                        ps = ps_b.tile([128, 2, 512], FP32, tag="ps_pv", bufs=2)
                        for eh in range(2):
                            for tf in range(FC // 2):
                                nc.tensor.matmul(
                                    ps[:, eh, 0:EH],
                                    hT8[:, 2 * tf:2 * tf + 2, sc * 128:(sc + 1) * 128],
                                    w2_t[:, 2 * tf:2 * tf + 2, eh * EH:(eh + 1) * EH],
                                    start=(tf == 0), stop=(tf == FC // 2 - 1),
                                    perf_mode=DR,
                                )
                        ypre = f2_pool.tile([128, D], FP32, tag="ypre", bufs=3)
                        nc.vector.scalar_tensor_tensor(
                            out=ypre[:], in0=ps[:, :, 0:EH], scalar=WS_INV,
                            in1=xn_big[:, sc, :], op0=OP.mult, op1=OP.add)
                        stats = f2_pool.tile([128, 2, 6], FP32, tag="st2")
                        nc.vector.bn_stats(stats[:, 0, :], ypre[:, 0:512])
                        nc.vector.bn_stats(stats[:, 1, :], ypre[:, 512:768])
                        mv = f2_pool.tile([128, 2], FP32, tag="mv2")
                        nc.vector.bn_aggr(mv[:], stats[:])
                        rstd2 = f2_pool.tile([128, 1], FP32, tag="rstd2")
                        nc.scalar.activation(
                            rstd2[:], mv[:, 1:2], AF.Ln, bias=eps_t[:])
                        nc.scalar.activation(
                            rstd2[:], rstd2[:], AF.Exp, scale=-0.5)
                        if defer2[0] is not None:
                            ln2_apply(*defer2[0])
                        defer2[0] = (sc, ypre, mv, rstd2)
                    ln2_apply(*defer2[0])
                es_late.close()

    _split_multi_waits(nc)
    return nc


_NC_CACHE = None


def kernel(**inputs):
    global _NC_CACHE
    if _NC_CACHE is None:
        _NC_CACHE = build_program()
    nc = _NC_CACHE

    f32 = lambda k: np.asarray(inputs[k], dtype=np.float32)

    # scores = q@k^T = src M src^T + u[q] + w[k] + c with M = wq wk^T,
    # u = src@(wq bk), w = src@(wk bq), c = bq.bk  (bias terms -> gmT)
    Mqk = f32("wq") @ f32("wk").T
    m8 = np.ascontiguousarray(
        (Mqk * WSM).reshape(DC, 128, D).transpose(1, 0, 2)).astype(F8NP)
    wv_bf = np.ascontiguousarray(
        f32("wv").reshape(DC, 128, D).transpose(1, 0, 2)).astype(BF16NP)
    # fold ln1 affine into FFN1 (exact): gelu((y*w+b)@w1 + b1)
    #   = gelu(y@(diag(w)w1) + (b1 + b@w1))
    w1f = f32("w1") * f32("ln1_w")[:, None]
    b1f = f32("b1") + f32("ln1_b") @ f32("w1")
    w1_bf = np.ascontiguousarray(
        w1f.reshape(DC, 128, DFF).transpose(1, 0, 2)).astype(BF16NP)
    w28 = np.ascontiguousarray(
        (f32("w2") * WS).reshape(FC, 128, D).transpose(1, 0, 2)).astype(F8NP)

    shared = {
        "m8": m8, "wv_bf": wv_bf, "w1_bf": w1_bf, "w28": w28,
        "bv": f32("bv").astype(BF16NP),
        "b1": np.ascontiguousarray(b1f.reshape(FC, 128).T),
        "ln1_w": f32("ln1_w").astype(BF16NP),
        "ln1b2": (f32("ln1_b") + f32("b2")).astype(BF16NP),
        "ln2_w": f32("ln2_w").astype(BF16NP),
        "ln2_b": f32("ln2_b").astype(BF16NP),
    }
    src = np.asarray(inputs["src"], dtype=np.float32)
    gum = np.asarray(inputs["gumbel_u"], dtype=np.float32)
    # gumbel + score bias terms: expin = (srcMsrc^T)*scale - gm' with
    # gm'[q,k] = ln(-ln u) - scale*(u[q] + w[k] + c); then transpose
    uq = src @ (f32("wq") @ f32("bk"))            # [B,S]
    wk_ = src @ (f32("wk") @ f32("bq"))           # [B,S]
    cc = float(f32("bq") @ f32("bk"))
    gm = np.log(-np.log(gum))
    gm -= SCALE * (uq[:, :, None] + wk_[:, None, :] + cc)
    gmT = gm.transpose(0, 2, 1)

    in_maps = []
    for c in range(N_CORES):
        m = dict(shared)
        sc_ = src[c * BL:(c + 1) * BL]
        m["src_bf"] = np.ascontiguousarray(sc_.astype(BF16NP))
        srcT = np.ascontiguousarray(
            sc_.reshape(BL, S, DC, 128).transpose(0, 3, 2, 1))
        m["srcT8"] = srcT.astype(F8NP)
        m["srcT_bf"] = srcT.astype(BF16NP)
        m["gmT"] = np.ascontiguousarray(
            gmT[c * BL:(c + 1) * BL]).astype(np.float16)
        in_maps.append(m)

    res = run_bass_kernel_spmd(nc, in_maps, core_ids=list(range(N_CORES)))
    return np.concatenate(
        [res.results[c]["out"].astype(np.float32) for c in range(N_CORES)],
        axis=0)
